# revision 2
# baseline (speedup 1.0000x reference)
"""Trainium2 Bass kernel v7 for MiniMemory: gated linear recurrence, bf16.

    mass  = sigmoid(x @ w_mass)            # [T]
    decay = sigmoid(x @ w_decay)           # [T]
    s_t   = decay_t * s_{t-1} + mass_t * x_t
    out   = s                              # [T, D]

Data-parallel over B across 8 NeuronCores; device I/O in bf16, host casts.
33 uniform blocks of L=127 timesteps in 128-row DRAM slots (full-tile DMAs
spray across all 16 DMA engines). Slot row 127 of each x tile receives the
previous block's last output row; the scan's augmented row turns it into
the carry term of the K=128 value matmul:
    out[f] = sum_{k=0..126} ATm[k, f] * m_k * x[k, :] + e[f] * carry
    ATaug[:, f] = d_f * ATaug[:, f-1] + I[:, f],  initial = I[:, 127]

The block-to-block carry is a serial loop (matmul -> eviction -> DMA ->
next matmul, ~4 us of latency). To hide it, time is split into FOUR
independent carry chains, each started by one redundant zero-carry warmup
block (the recurrence's memory decays as prod sigmoid(logits) ~ e^-0.8/step,
so 127 warmup steps reproduce the running state exactly to fp32). Rounds
interleave one block from each chain, so every chain has ~4 block-times of
slack for its carry hop and the tensor engine stays busy (HAM clock warm).

Gates run on the tensor engine per round of 4 blocks: PE transposes of x
chunks (rows 0..126 only - the gate path never waits on a carry), 16
accumulating N=512 gate matmuls against packed w2, one sigmoid per round.
Dead logit columns (carry slots) are pinned to +30 so the carry's mass
scale is exactly 1. ATm is built 128 columns wide (column 127 dead) so
the value matmuls get fast weight load. Bulk DMAs issue from the Sync
hardware queue; the tiny carry DMAs from GPSIMD's queue.
"""

import numpy as np


def _ensure_path():
    try:
        import concourse.bass_utils  # noqa: F401
    except ImportError:
        import sys
        for p in ("/opt/trn_rl_repo", "/root/.axon_site/_ro/trn_rl_repo"):
            if p not in sys.path:
                sys.path.insert(0, p)
        import concourse.bass_utils  # noqa: F401


_ensure_path()

import ml_dtypes  # noqa: E402
import concourse.bacc as bacc  # noqa: E402
import concourse.tile as tile  # noqa: E402
from concourse import mybir  # noqa: E402
from concourse.bass_utils import run_bass_kernel_spmd  # noqa: E402
from concourse.masks import make_identity  # noqa: E402

B, T, D = 8, 4096, 2048
L = 127                      # valid timesteps per block (slot 127 = carry)
NBLK = (T + L - 1) // L      # 33
NCHUNK = D // 128
NCORES = 8
NCHAIN = 4
F32 = mybir.dt.float32
BF16 = mybir.dt.bfloat16
AF = mybir.ActivationFunctionType
ALU = mybir.AluOpType


def _chain_schedule():
    """Jobs grouped into rounds; each job: (dram_block, has_out)."""
    starts = [0, 8, 16, 24]
    chains = []
    for j, s in enumerate(starts):
        end = starts[j + 1] if j + 1 < NCHAIN else NBLK
        ch = []
        if j > 0:
            ch.append((s - 1, False))          # zero-carry warmup block
        ch += [(b, True) for b in range(s, end)]
        chains.append(ch)
    rounds = []
    r = 0
    while True:
        rnd = [(j, chains[j][r]) for j in range(NCHAIN) if r < len(chains[j])]
        if not rnd:
            break
        rounds.append(rnd)
        r += 1
    return chains, rounds


def build_kernel():
    chains, rounds = _chain_schedule()

    nc = bacc.Bacc("TRN2", target_bir_lowering=False, debug=False)
    x_d = nc.dram_tensor("x", [NBLK * 128, D], BF16,
                         kind="ExternalInput").ap()
    w2_d = nc.dram_tensor("w2", [128, 2 * NCHUNK], BF16,
                          kind="ExternalInput").ap()
    out_d = nc.dram_tensor("out", [NBLK * 128, D], BF16,
                           kind="ExternalOutput").ap()

    with tile.TileContext(nc) as tc:
        with (
            tc.tile_pool(name="consts", bufs=1) as consts,
            tc.tile_pool(name="xp", bufs=12) as xp,
            tc.tile_pool(name="xtp", bufs=3) as xtp,
            tc.tile_pool(name="op", bufs=4) as op,
            tc.tile_pool(name="gp", bufs=2) as gp,
            tc.tile_pool(name="small", bufs=5) as small,
            tc.tile_pool(name="psT", bufs=2, space="PSUM") as psT,
            tc.tile_pool(name="psO", bufs=3, space="PSUM") as psO,
            tc.tile_pool(name="psG", bufs=1, space="PSUM") as psG,
            tc.tile_pool(name="psGc", bufs=1, space="PSUM") as psGc,
            tc.tile_pool(name="psDb", bufs=1, space="PSUM") as psDb,
        ):
            identb = consts.tile([128, 128], BF16)
            make_identity(nc, identb)
            identf = consts.tile([128, 128], F32)
            make_identity(nc, identf)
            ones_row = consts.tile([1, 128], BF16)
            nc.vector.memset(ones_row, 1.0)
            w2 = consts.tile([128, 2 * NCHUNK], BF16)
            nc.sync.dma_start(out=w2, in_=w2_d)

            # per-job x tiles keyed by (chain, pos)
            xt_of = {}
            evrot = [0]

            def load_x(j, p):
                """Load x tile for chain j position p (idempotent)."""
                if p >= len(chains[j]) or (j, p) in xt_of:
                    return
                b = chains[j][p][0]
                t = xp.tile([128, D], BF16, tag="xa", name=f"xa{j}_{p}")
                nc.sync.dma_start(out=t, in_=x_d[b * 128:(b + 1) * 128, :])
                xt_of[(j, p)] = t

            def evict(out, in_):
                r = evrot[0] % 2
                evrot[0] += 1
                if r == 0:
                    nc.scalar.activation(out=out, in_=in_, func=AF.Copy)
                else:
                    nc.vector.tensor_copy(out=out, in_=in_)

            for j in range(NCHAIN):
                load_x(j, 0)

            for r, rnd in enumerate(rounds):
                jobs = [(j, r, b, has_out) for j, (b, has_out) in rnd]

                # ---- gate phase for this round's blocks ----
                gps = psG.tile([2, 512], F32, tag="g", name=f"gps{r}")
                xT = [None] * (NCHUNK // 2)
                for pp in range(NCHUNK // 2):
                    pT = psT.tile([128, 1024], BF16, tag="pt",
                                  name=f"pT{r}_{pp}")
                    for h in range(2):
                        c = 2 * pp + h
                        for idx, (j, p, b, has_out) in enumerate(jobs):
                            nc.tensor.transpose(
                                out=pT[:, h * 512 + idx * 128:
                                       h * 512 + idx * 128 + L],
                                in_=xt_of[(j, p)][0:L,
                                                  c * 128:(c + 1) * 128],
                                identity=identb[0:L, 0:L])
                    xT[pp] = xtp.tile([128, 1024], BF16, tag="xt",
                                      name=f"xT{r}_{pp}")
                    evict(xT[pp], pT)
                for pp in range(NCHUNK // 2):
                    for h in range(2):
                        c = 2 * pp + h
                        nc.tensor.matmul(
                            gps, lhsT=w2[:, 2 * c:2 * c + 2],
                            rhs=xT[pp][:, h * 512:(h + 1) * 512],
                            start=(c == 0), stop=(c == NCHUNK - 1))
                for idx in range(len(jobs)):
                    nc.vector.memset(
                        gps[0:2, idx * 128 + 127:idx * 128 + 128], 30.0)
                gsig = gp.tile([2, 512], BF16, tag="gsig", name=f"gsig{r}")
                nc.scalar.activation(out=gsig, in_=gps, func=AF.Sigmoid)
                dbc = psDb.tile([128, 512], F32, tag="dbc", name=f"dbc{r}")
                nc.tensor.matmul(dbc, lhsT=ones_row, rhs=gsig[0:1, :],
                                 start=True, stop=True)

                # loads for the next round (must precede this round's
                # carry DMAs into those tiles)
                for j, p, b, has_out in jobs:
                    load_x(j, p + 1)

                # ---- per-block A-matrix + value ----
                for idx, (j, p, b, has_out) in enumerate(jobs):
                    xa = xt_of[(j, p)]
                    gc_ps = psGc.tile([128, 2], F32, tag="gc",
                                      name=f"gc{r}_{idx}")
                    nc.tensor.matmul(
                        gc_ps, lhsT=gsig[:, idx * 128:(idx + 1) * 128],
                        rhs=identb[0:2, 0:2], start=True, stop=True)
                    gcols = small.tile([128, 2], F32, tag="gcols",
                                       name=f"gcols{r}_{idx}")
                    nc.scalar.activation(out=gcols, in_=gc_ps,
                                         func=AF.Copy)

                    # 128 cols: col 127 dead, enables fast weight load
                    ATaug = small.tile([128, 128], F32, tag="ATaug",
                                       name=f"ATaug{r}_{idx}")
                    nc.vector.tensor_tensor_scan(
                        out=ATaug,
                        data0=dbc[:, idx * 128:(idx + 1) * 128],
                        data1=identf,
                        initial=identf[:, 127:128],
                        op0=ALU.mult, op1=ALU.add)
                    ATm = small.tile([128, 128], BF16, tag="ATm",
                                     name=f"ATm{r}_{idx}")
                    nc.vector.tensor_scalar_mul(ATm, ATaug, gcols[:, 1:2])

                    out_sb = op.tile([128, D], BF16, tag="o",
                                     name=f"o{r}_{idx}")
                    for s in range(4):
                        sl = slice(s * 512, (s + 1) * 512)
                        ops = psO.tile([128, 512], F32, tag="psO",
                                       name=f"ops{r}_{idx}_{s}")
                        nc.tensor.matmul(ops, lhsT=ATm, rhs=xa[:, sl],
                                         start=True, stop=True)
                        evict(out_sb[:, sl], ops)
                    if has_out:
                        nc.sync.dma_start(
                            out=out_d[b * 128:(b + 1) * 128, :],
                            in_=out_sb)
                    # carry to the next block in this chain
                    if p + 1 < len(chains[j]):
                        nc.gpsimd.dma_start(
                            out=xt_of[(j, p + 1)][127:128, :],
                            in_=out_sb[L - 1:L, :])
    nc.compile()
    return nc


def pack_w2(w_mass, w_decay):
    w2 = np.empty((128, 2 * NCHUNK), dtype=ml_dtypes.bfloat16)
    wm = np.asarray(w_mass, np.float32).reshape(NCHUNK, 128)
    wd = np.asarray(w_decay, np.float32).reshape(NCHUNK, 128)
    w2[:, 0::2] = wd.T.astype(ml_dtypes.bfloat16)
    w2[:, 1::2] = wm.T.astype(ml_dtypes.bfloat16)
    return np.ascontiguousarray(w2)


def pad_x(xi):
    """[T, D] fp32 -> [NBLK*128, D] bf16 block-slot layout (row 127 = 0)."""
    xb = np.zeros((NBLK * 128, D), dtype=ml_dtypes.bfloat16)
    flat = np.zeros((NBLK * L, D), dtype=ml_dtypes.bfloat16)
    flat[0:T] = xi.astype(ml_dtypes.bfloat16)
    xb.reshape(NBLK, 128, D)[:, 0:L, :] = flat.reshape(NBLK, L, D)
    return xb


_CACHE = {}


def _get_nc():
    if "nc" not in _CACHE:
        _CACHE["nc"] = build_kernel()
    return _CACHE["nc"]


def make_in_maps(x, w_mass, w_decay):
    x = np.asarray(x, np.float32)
    w2 = pack_w2(w_mass, w_decay)
    return [{"x": pad_x(x[i]), "w2": w2} for i in range(B)]


def post_process(res):
    outs = []
    for i in range(B):
        o = res.results[i]["out"].reshape(NBLK, 128, D)[:, 0:L, :]
        outs.append(o.reshape(NBLK * L, D)[0:T].astype(np.float32))
    return np.stack(outs, axis=0)


def kernel(x, w_mass, w_decay):
    nc = _get_nc()
    in_maps = make_in_maps(x, w_mass, w_decay)
    res = run_bass_kernel_spmd(nc, in_maps, core_ids=list(range(NCORES)))
    return post_process(res)


# revision 3
# speedup vs baseline: 1.0923x; 1.0923x over previous
"""Trainium2 Bass kernel v7 for MiniMemory: gated linear recurrence, bf16.

    mass  = sigmoid(x @ w_mass)            # [T]
    decay = sigmoid(x @ w_decay)           # [T]
    s_t   = decay_t * s_{t-1} + mass_t * x_t
    out   = s                              # [T, D]

Data-parallel over B across 8 NeuronCores; device I/O in bf16, host casts.
33 uniform blocks of L=127 timesteps in 128-row DRAM slots (full-tile DMAs
spray across all 16 DMA engines). Slot row 127 of each x tile receives the
previous block's last output row; the scan's augmented row turns it into
the carry term of the K=128 value matmul:
    out[f] = sum_{k=0..126} ATm[k, f] * m_k * x[k, :] + e[f] * carry
    ATaug[:, f] = d_f * ATaug[:, f-1] + I[:, f],  initial = I[:, 127]

The block-to-block carry is a serial loop (matmul -> eviction -> DMA ->
next matmul, ~4 us of latency). To hide it, time is split into FOUR
independent carry chains, each started by one redundant zero-carry warmup
block (the recurrence's memory decays as prod sigmoid(logits) ~ e^-0.8/step,
so 127 warmup steps reproduce the running state exactly to fp32). Rounds
interleave one block from each chain, so every chain has ~4 block-times of
slack for its carry hop and the tensor engine stays busy (HAM clock warm).

Gates run on the tensor engine per round of 4 blocks: PE transposes of x
chunks (rows 0..126 only - the gate path never waits on a carry), 16
accumulating N=512 gate matmuls against packed w2, one sigmoid per round.
Dead logit columns (carry slots) are pinned to +30 so the carry's mass
scale is exactly 1. ATm is built 128 columns wide (column 127 dead) so
the value matmuls get fast weight load. Bulk DMAs issue from the Sync
hardware queue; the tiny carry DMAs from GPSIMD's queue.
"""

import numpy as np


def _ensure_path():
    try:
        import concourse.bass_utils  # noqa: F401
    except ImportError:
        import sys
        for p in ("/opt/trn_rl_repo", "/root/.axon_site/_ro/trn_rl_repo"):
            if p not in sys.path:
                sys.path.insert(0, p)
        import concourse.bass_utils  # noqa: F401


_ensure_path()

import ml_dtypes  # noqa: E402
import concourse.bacc as bacc  # noqa: E402
import concourse.tile as tile  # noqa: E402
from concourse import mybir  # noqa: E402
from concourse.bass_utils import run_bass_kernel_spmd  # noqa: E402
from concourse.masks import make_identity  # noqa: E402

B, T, D = 8, 4096, 2048
L = 127                      # valid timesteps per block (slot 127 = carry)
NBLK = (T + L - 1) // L      # 33
NCHUNK = D // 128
NCORES = 8
NCHAIN = 4
F32 = mybir.dt.float32
BF16 = mybir.dt.bfloat16
AF = mybir.ActivationFunctionType
ALU = mybir.AluOpType


def _chain_schedule():
    """Jobs grouped into rounds; each job: (dram_block, has_out)."""
    starts = [0, 9, 17, 25]
    chains = []
    for j, s in enumerate(starts):
        end = starts[j + 1] if j + 1 < NCHAIN else NBLK
        ch = []
        if j > 0:
            ch.append((s - 1, False))          # zero-carry warmup block
        ch += [(b, True) for b in range(s, end)]
        chains.append(ch)
    rounds = []
    r = 0
    while True:
        rnd = [(j, chains[j][r]) for j in range(NCHAIN) if r < len(chains[j])]
        if not rnd:
            break
        rounds.append(rnd)
        r += 1
    return chains, rounds


def build_kernel():
    chains, rounds = _chain_schedule()

    nc = bacc.Bacc("TRN2", target_bir_lowering=False, debug=False)
    x_d = nc.dram_tensor("x", [NBLK * 128, D], BF16,
                         kind="ExternalInput").ap()
    w2_d = nc.dram_tensor("w2", [128, 2 * NCHUNK], BF16,
                          kind="ExternalInput").ap()
    out_d = nc.dram_tensor("out", [NBLK * 128, D], BF16,
                           kind="ExternalOutput").ap()

    with tile.TileContext(nc) as tc:
        with (
            tc.tile_pool(name="consts", bufs=1) as consts,
            tc.tile_pool(name="xp", bufs=12) as xp,
            tc.tile_pool(name="xtp", bufs=3) as xtp,
            tc.tile_pool(name="op", bufs=4) as op,
            tc.tile_pool(name="gp", bufs=2) as gp,
            tc.tile_pool(name="small", bufs=5) as small,
            tc.tile_pool(name="psT", bufs=2, space="PSUM") as psT,
            tc.tile_pool(name="psO", bufs=3, space="PSUM") as psO,
            tc.tile_pool(name="psG", bufs=1, space="PSUM") as psG,
            tc.tile_pool(name="psGc", bufs=1, space="PSUM") as psGc,
            tc.tile_pool(name="psDb", bufs=1, space="PSUM") as psDb,
        ):
            identb = consts.tile([128, 128], BF16)
            make_identity(nc, identb)
            identf = consts.tile([128, 128], F32)
            make_identity(nc, identf)
            ones_row = consts.tile([1, 128], BF16)
            nc.vector.memset(ones_row, 1.0)
            w2 = consts.tile([128, 2 * NCHUNK], BF16)
            nc.sync.dma_start(out=w2, in_=w2_d)

            # per-job x tiles keyed by (chain, pos)
            xt_of = {}
            evrot = [0]

            def load_x(j, p):
                """Load x tile for chain j position p (idempotent)."""
                if p >= len(chains[j]) or (j, p) in xt_of:
                    return
                b = chains[j][p][0]
                t = xp.tile([128, D], BF16, tag="xa", name=f"xa{j}_{p}")
                nc.sync.dma_start(out=t, in_=x_d[b * 128:(b + 1) * 128, :])
                xt_of[(j, p)] = t

            def evict(out, in_):
                r = evrot[0] % 2
                evrot[0] += 1
                if r == 0:
                    nc.scalar.activation(out=out, in_=in_, func=AF.Copy)
                else:
                    nc.vector.tensor_copy(out=out, in_=in_)

            for j in range(NCHAIN):
                load_x(j, 0)

            for r, rnd in enumerate(rounds):
                jobs = [(j, r, b, has_out) for j, (b, has_out) in rnd]

                # ---- gate phase for this round's blocks ----
                gps = psG.tile([2, 512], F32, tag="g", name=f"gps{r}")
                xT = [None] * (NCHUNK // 2)
                for pp in range(NCHUNK // 2):
                    pT = psT.tile([128, 1024], BF16, tag="pt",
                                  name=f"pT{r}_{pp}")
                    for h in range(2):
                        c = 2 * pp + h
                        for idx, (j, p, b, has_out) in enumerate(jobs):
                            nc.tensor.transpose(
                                out=pT[:, h * 512 + idx * 128:
                                       h * 512 + idx * 128 + L],
                                in_=xt_of[(j, p)][0:L,
                                                  c * 128:(c + 1) * 128],
                                identity=identb[0:L, 0:L])
                    xT[pp] = xtp.tile([128, 1024], BF16, tag="xt",
                                      name=f"xT{r}_{pp}")
                    evict(xT[pp], pT)
                for pp in range(NCHUNK // 2):
                    for h in range(2):
                        c = 2 * pp + h
                        nc.tensor.matmul(
                            gps, lhsT=w2[:, 2 * c:2 * c + 2],
                            rhs=xT[pp][:, h * 512:(h + 1) * 512],
                            start=(c == 0), stop=(c == NCHUNK - 1))
                for idx in range(len(jobs)):
                    nc.vector.memset(
                        gps[0:2, idx * 128 + 127:idx * 128 + 128], 30.0)
                gsig = gp.tile([2, 512], BF16, tag="gsig", name=f"gsig{r}")
                nc.scalar.activation(out=gsig, in_=gps, func=AF.Sigmoid)
                dbc = psDb.tile([128, 512], F32, tag="dbc", name=f"dbc{r}")
                nc.tensor.matmul(dbc, lhsT=ones_row, rhs=gsig[0:1, :],
                                 start=True, stop=True)

                # loads for the next round (must precede this round's
                # carry DMAs into those tiles)
                for j, p, b, has_out in jobs:
                    load_x(j, p + 1)

                # ---- per-block A-matrix + value ----
                for idx, (j, p, b, has_out) in enumerate(jobs):
                    xa = xt_of[(j, p)]
                    gc_ps = psGc.tile([128, 2], F32, tag="gc",
                                      name=f"gc{r}_{idx}")
                    nc.tensor.matmul(
                        gc_ps, lhsT=gsig[:, idx * 128:(idx + 1) * 128],
                        rhs=identb[0:2, 0:2], start=True, stop=True)
                    gcols = small.tile([128, 2], F32, tag="gcols",
                                       name=f"gcols{r}_{idx}")
                    nc.scalar.activation(out=gcols, in_=gc_ps,
                                         func=AF.Copy)

                    # 128 cols: col 127 dead, enables fast weight load
                    ATaug = small.tile([128, 128], F32, tag="ATaug",
                                       name=f"ATaug{r}_{idx}")
                    nc.vector.tensor_tensor_scan(
                        out=ATaug,
                        data0=dbc[:, idx * 128:(idx + 1) * 128],
                        data1=identf,
                        initial=identf[:, 127:128],
                        op0=ALU.mult, op1=ALU.add)
                    ATm = small.tile([128, 128], BF16, tag="ATm",
                                     name=f"ATm{r}_{idx}")
                    nc.vector.tensor_scalar_mul(ATm, ATaug, gcols[:, 1:2])

                    out_sb = op.tile([128, D], BF16, tag="o",
                                     name=f"o{r}_{idx}")
                    for s in range(4):
                        sl = slice(s * 512, (s + 1) * 512)
                        ops = psO.tile([128, 512], F32, tag="psO",
                                       name=f"ops{r}_{idx}_{s}")
                        nc.tensor.matmul(ops, lhsT=ATm, rhs=xa[:, sl],
                                         start=True, stop=True)
                        evict(out_sb[:, sl], ops)
                    if has_out:
                        nc.sync.dma_start(
                            out=out_d[b * 128:(b + 1) * 128, :],
                            in_=out_sb)
                    # carry to the next block in this chain
                    if p + 1 < len(chains[j]):
                        nc.gpsimd.dma_start(
                            out=xt_of[(j, p + 1)][127:128, :],
                            in_=out_sb[L - 1:L, :])
    nc.compile()
    return nc


def pack_w2(w_mass, w_decay):
    w2 = np.empty((128, 2 * NCHUNK), dtype=ml_dtypes.bfloat16)
    wm = np.asarray(w_mass, np.float32).reshape(NCHUNK, 128)
    wd = np.asarray(w_decay, np.float32).reshape(NCHUNK, 128)
    w2[:, 0::2] = wd.T.astype(ml_dtypes.bfloat16)
    w2[:, 1::2] = wm.T.astype(ml_dtypes.bfloat16)
    return np.ascontiguousarray(w2)


def pad_x(xi):
    """[T, D] fp32 -> [NBLK*128, D] bf16 block-slot layout (row 127 = 0)."""
    xb = np.zeros((NBLK * 128, D), dtype=ml_dtypes.bfloat16)
    flat = np.zeros((NBLK * L, D), dtype=ml_dtypes.bfloat16)
    flat[0:T] = xi.astype(ml_dtypes.bfloat16)
    xb.reshape(NBLK, 128, D)[:, 0:L, :] = flat.reshape(NBLK, L, D)
    return xb


_CACHE = {}


def _get_nc():
    if "nc" not in _CACHE:
        _CACHE["nc"] = build_kernel()
    return _CACHE["nc"]


def make_in_maps(x, w_mass, w_decay):
    x = np.asarray(x, np.float32)
    w2 = pack_w2(w_mass, w_decay)
    return [{"x": pad_x(x[i]), "w2": w2} for i in range(B)]


def post_process(res):
    outs = []
    for i in range(B):
        o = res.results[i]["out"].reshape(NBLK, 128, D)[:, 0:L, :]
        outs.append(o.reshape(NBLK * L, D)[0:T].astype(np.float32))
    return np.stack(outs, axis=0)


def kernel(x, w_mass, w_decay):
    nc = _get_nc()
    in_maps = make_in_maps(x, w_mass, w_decay)
    res = run_bass_kernel_spmd(nc, in_maps, core_ids=list(range(NCORES)))
    return post_process(res)


# revision 4
# speedup vs baseline: 1.1122x; 1.0181x over previous
"""Trainium2 Bass kernel v7 for MiniMemory: gated linear recurrence, bf16.

    mass  = sigmoid(x @ w_mass)            # [T]
    decay = sigmoid(x @ w_decay)           # [T]
    s_t   = decay_t * s_{t-1} + mass_t * x_t
    out   = s                              # [T, D]

Data-parallel over B across 8 NeuronCores; device I/O in bf16, host casts.
33 uniform blocks of L=127 timesteps in 128-row DRAM slots (full-tile DMAs
spray across all 16 DMA engines). Slot row 127 of each x tile receives the
previous block's last output row; the scan's augmented row turns it into
the carry term of the K=128 value matmul:
    out[f] = sum_{k=0..126} ATm[k, f] * m_k * x[k, :] + e[f] * carry
    ATaug[:, f] = d_f * ATaug[:, f-1] + I[:, f],  initial = I[:, 127]

The block-to-block carry is a serial loop (matmul -> eviction -> DMA ->
next matmul, ~4 us of latency). To hide it, time is split into FOUR
independent carry chains, each started by one redundant zero-carry warmup
block (the recurrence's memory decays as prod sigmoid(logits) ~ e^-0.8/step,
so 127 warmup steps reproduce the running state exactly to fp32). Rounds
interleave one block from each chain, so every chain has ~4 block-times of
slack for its carry hop and the tensor engine stays busy (HAM clock warm).

Gates run on the tensor engine per round of 4 blocks: PE transposes of x
chunks (rows 0..126 only - the gate path never waits on a carry), 16
accumulating N=512 gate matmuls against packed w2, one sigmoid per round.
Dead logit columns (carry slots) are pinned to +30 so the carry's mass
scale is exactly 1. ATm is built 128 columns wide (column 127 dead) so
the value matmuls get fast weight load. Bulk DMAs issue from the Sync
hardware queue; the tiny carry DMAs from GPSIMD's queue.
"""

import numpy as np


def _ensure_path():
    try:
        import concourse.bass_utils  # noqa: F401
    except ImportError:
        import sys
        for p in ("/opt/trn_rl_repo", "/root/.axon_site/_ro/trn_rl_repo"):
            if p not in sys.path:
                sys.path.insert(0, p)
        import concourse.bass_utils  # noqa: F401


_ensure_path()

import ml_dtypes  # noqa: E402
import concourse.bacc as bacc  # noqa: E402
import concourse.tile as tile  # noqa: E402
from concourse import mybir  # noqa: E402
from concourse.bass_utils import run_bass_kernel_spmd  # noqa: E402
from concourse.masks import make_identity  # noqa: E402

B, T, D = 8, 4096, 2048
L = 127                      # valid timesteps per block (slot 127 = carry)
NBLK = (T + L - 1) // L      # 33
NCHUNK = D // 128
NCORES = 8
NCHAIN = 4
F32 = mybir.dt.float32
BF16 = mybir.dt.bfloat16
AF = mybir.ActivationFunctionType
ALU = mybir.AluOpType


def _chain_schedule():
    """Jobs grouped into rounds; each job: (dram_block, has_out)."""
    starts = [0, 9, 17, 25]
    chains = []
    for j, s in enumerate(starts):
        end = starts[j + 1] if j + 1 < NCHAIN else NBLK
        ch = []
        if j > 0:
            ch.append((s - 1, False))          # zero-carry warmup block
        ch += [(b, True) for b in range(s, end)]
        chains.append(ch)
    rounds = []
    r = 0
    while True:
        rnd = [(j, chains[j][r]) for j in range(NCHAIN) if r < len(chains[j])]
        if not rnd:
            break
        rounds.append(rnd)
        r += 1
    return chains, rounds


def build_kernel():
    chains, rounds = _chain_schedule()

    nc = bacc.Bacc("TRN2", target_bir_lowering=False, debug=False)
    x_d = nc.dram_tensor("x", [NBLK * 128, D], BF16,
                         kind="ExternalInput").ap()
    w2_d = nc.dram_tensor("w2", [128, 2 * NCHUNK], BF16,
                          kind="ExternalInput").ap()
    out_d = nc.dram_tensor("out", [NBLK * 128, D], BF16,
                           kind="ExternalOutput").ap()

    with tile.TileContext(nc) as tc:
        with (
            tc.tile_pool(name="consts", bufs=1) as consts,
            tc.tile_pool(name="xp", bufs=14) as xp,
            tc.tile_pool(name="xtp", bufs=4) as xtp,
            tc.tile_pool(name="op", bufs=5) as op,
            tc.tile_pool(name="gp", bufs=3) as gp,
            tc.tile_pool(name="small", bufs=8) as small,
            tc.tile_pool(name="psT", bufs=2, space="PSUM") as psT,
            tc.tile_pool(name="psO", bufs=3, space="PSUM") as psO,
            tc.tile_pool(name="psG", bufs=1, space="PSUM") as psG,
            tc.tile_pool(name="psGc", bufs=1, space="PSUM") as psGc,
            tc.tile_pool(name="psDb", bufs=1, space="PSUM") as psDb,
        ):
            identb = consts.tile([128, 128], BF16)
            make_identity(nc, identb)
            identf = consts.tile([128, 128], F32)
            make_identity(nc, identf)
            ones_row = consts.tile([1, 128], BF16)
            nc.vector.memset(ones_row, 1.0)
            w2 = consts.tile([128, 2 * NCHUNK], BF16)
            nc.sync.dma_start(out=w2, in_=w2_d)

            # per-job x tiles keyed by (chain, pos)
            xt_of = {}
            evrot = [0]

            def load_x(j, p):
                """Load x tile for chain j position p (idempotent)."""
                if p >= len(chains[j]) or (j, p) in xt_of:
                    return
                b = chains[j][p][0]
                t = xp.tile([128, D], BF16, tag="xa", name=f"xa{j}_{p}")
                nc.sync.dma_start(out=t, in_=x_d[b * 128:(b + 1) * 128, :])
                xt_of[(j, p)] = t

            def evict(out, in_):
                r = evrot[0] % 2
                evrot[0] += 1
                if r == 0:
                    nc.scalar.activation(out=out, in_=in_, func=AF.Copy)
                else:
                    nc.vector.tensor_copy(out=out, in_=in_)

            for j in range(NCHAIN):
                load_x(j, 0)

            for r, rnd in enumerate(rounds):
                jobs = [(j, r, b, has_out) for j, (b, has_out) in rnd]

                # ---- gate phase for this round's blocks ----
                gps = psG.tile([2, 512], F32, tag="g", name=f"gps{r}")
                xT = [None] * (NCHUNK // 2)
                for pp in range(NCHUNK // 2):
                    pT = psT.tile([128, 1024], BF16, tag="pt",
                                  name=f"pT{r}_{pp}")
                    for h in range(2):
                        c = 2 * pp + h
                        for idx, (j, p, b, has_out) in enumerate(jobs):
                            nc.tensor.transpose(
                                out=pT[:, h * 512 + idx * 128:
                                       h * 512 + idx * 128 + L],
                                in_=xt_of[(j, p)][0:L,
                                                  c * 128:(c + 1) * 128],
                                identity=identb[0:L, 0:L])
                    xT[pp] = xtp.tile([128, 1024], BF16, tag="xt",
                                      name=f"xT{r}_{pp}")
                    evict(xT[pp], pT)
                for pp in range(NCHUNK // 2):
                    for h in range(2):
                        c = 2 * pp + h
                        nc.tensor.matmul(
                            gps, lhsT=w2[:, 2 * c:2 * c + 2],
                            rhs=xT[pp][:, h * 512:(h + 1) * 512],
                            start=(c == 0), stop=(c == NCHUNK - 1))
                for idx in range(len(jobs)):
                    nc.vector.memset(
                        gps[0:2, idx * 128 + 127:idx * 128 + 128], 30.0)
                gsig = gp.tile([2, 512], BF16, tag="gsig", name=f"gsig{r}")
                nc.scalar.activation(out=gsig, in_=gps, func=AF.Sigmoid)
                dbc = psDb.tile([128, 512], F32, tag="dbc", name=f"dbc{r}")
                nc.tensor.matmul(dbc, lhsT=ones_row, rhs=gsig[0:1, :],
                                 start=True, stop=True)

                # loads for the next round (must precede this round's
                # carry DMAs into those tiles)
                for j, p, b, has_out in jobs:
                    load_x(j, p + 1)

                # ---- per-block A-matrix + value ----
                for idx, (j, p, b, has_out) in enumerate(jobs):
                    xa = xt_of[(j, p)]
                    gc_ps = psGc.tile([128, 2], F32, tag="gc",
                                      name=f"gc{r}_{idx}")
                    nc.tensor.matmul(
                        gc_ps, lhsT=gsig[:, idx * 128:(idx + 1) * 128],
                        rhs=identb[0:2, 0:2], start=True, stop=True)
                    gcols = small.tile([128, 2], F32, tag="gcols",
                                       name=f"gcols{r}_{idx}")
                    nc.scalar.activation(out=gcols, in_=gc_ps,
                                         func=AF.Copy)

                    # 128 cols: col 127 dead, enables fast weight load
                    ATaug = small.tile([128, 128], F32, tag="ATaug",
                                       name=f"ATaug{r}_{idx}")
                    nc.vector.tensor_tensor_scan(
                        out=ATaug,
                        data0=dbc[:, idx * 128:(idx + 1) * 128],
                        data1=identf,
                        initial=identf[:, 127:128],
                        op0=ALU.mult, op1=ALU.add)
                    ATm = small.tile([128, 128], BF16, tag="ATm",
                                     name=f"ATm{r}_{idx}")
                    nc.vector.tensor_scalar_mul(ATm, ATaug, gcols[:, 1:2])

                    out_sb = op.tile([128, D], BF16, tag="o",
                                     name=f"o{r}_{idx}")
                    for s in range(4):
                        sl = slice(s * 512, (s + 1) * 512)
                        ops = psO.tile([128, 512], F32, tag="psO",
                                       name=f"ops{r}_{idx}_{s}")
                        nc.tensor.matmul(ops, lhsT=ATm, rhs=xa[:, sl],
                                         start=True, stop=True)
                        evict(out_sb[:, sl], ops)
                    if has_out:
                        nc.sync.dma_start(
                            out=out_d[b * 128:(b + 1) * 128, :],
                            in_=out_sb)
                    # carry to the next block in this chain
                    if p + 1 < len(chains[j]):
                        nc.gpsimd.dma_start(
                            out=xt_of[(j, p + 1)][127:128, :],
                            in_=out_sb[L - 1:L, :])
    nc.compile()
    return nc


def pack_w2(w_mass, w_decay):
    w2 = np.empty((128, 2 * NCHUNK), dtype=ml_dtypes.bfloat16)
    wm = np.asarray(w_mass, np.float32).reshape(NCHUNK, 128)
    wd = np.asarray(w_decay, np.float32).reshape(NCHUNK, 128)
    w2[:, 0::2] = wd.T.astype(ml_dtypes.bfloat16)
    w2[:, 1::2] = wm.T.astype(ml_dtypes.bfloat16)
    return np.ascontiguousarray(w2)


def pad_x(xi):
    """[T, D] fp32 -> [NBLK*128, D] bf16 block-slot layout (row 127 = 0)."""
    xb = np.zeros((NBLK * 128, D), dtype=ml_dtypes.bfloat16)
    flat = np.zeros((NBLK * L, D), dtype=ml_dtypes.bfloat16)
    flat[0:T] = xi.astype(ml_dtypes.bfloat16)
    xb.reshape(NBLK, 128, D)[:, 0:L, :] = flat.reshape(NBLK, L, D)
    return xb


_CACHE = {}


def _get_nc():
    if "nc" not in _CACHE:
        _CACHE["nc"] = build_kernel()
    return _CACHE["nc"]


def make_in_maps(x, w_mass, w_decay):
    x = np.asarray(x, np.float32)
    w2 = pack_w2(w_mass, w_decay)
    return [{"x": pad_x(x[i]), "w2": w2} for i in range(B)]


def post_process(res):
    outs = []
    for i in range(B):
        o = res.results[i]["out"].reshape(NBLK, 128, D)[:, 0:L, :]
        outs.append(o.reshape(NBLK * L, D)[0:T].astype(np.float32))
    return np.stack(outs, axis=0)


def kernel(x, w_mass, w_decay):
    nc = _get_nc()
    in_maps = make_in_maps(x, w_mass, w_decay)
    res = run_bass_kernel_spmd(nc, in_maps, core_ids=list(range(NCORES)))
    return post_process(res)


# revision 5
# speedup vs baseline: 1.1205x; 1.0075x over previous
"""Trainium2 Bass kernel v7 for MiniMemory: gated linear recurrence, bf16.

    mass  = sigmoid(x @ w_mass)            # [T]
    decay = sigmoid(x @ w_decay)           # [T]
    s_t   = decay_t * s_{t-1} + mass_t * x_t
    out   = s                              # [T, D]

Data-parallel over B across 8 NeuronCores; device I/O in bf16, host casts.
33 uniform blocks of L=127 timesteps in 128-row DRAM slots (full-tile DMAs
spray across all 16 DMA engines). Slot row 127 of each x tile receives the
previous block's last output row; the scan's augmented row turns it into
the carry term of the K=128 value matmul:
    out[f] = sum_{k=0..126} ATm[k, f] * m_k * x[k, :] + e[f] * carry
    ATaug[:, f] = d_f * ATaug[:, f-1] + I[:, f],  initial = I[:, 127]

The block-to-block carry is a serial loop (matmul -> eviction -> DMA ->
next matmul, ~4 us of latency). To hide it, time is split into FOUR
independent carry chains, each started by one redundant zero-carry warmup
block (the recurrence's memory decays as prod sigmoid(logits) ~ e^-0.8/step,
so 127 warmup steps reproduce the running state exactly to fp32). Rounds
interleave one block from each chain, so every chain has ~4 block-times of
slack for its carry hop and the tensor engine stays busy (HAM clock warm).

Gates run on the tensor engine per round of 4 blocks: PE transposes of x
chunks (rows 0..126 only - the gate path never waits on a carry), 16
accumulating N=512 gate matmuls against packed w2, one sigmoid per round.
Dead logit columns (carry slots) are pinned to +30 so the carry's mass
scale is exactly 1. ATm is built 128 columns wide (column 127 dead) so
the value matmuls get fast weight load. Bulk DMAs issue from the Sync
hardware queue; the tiny carry DMAs from GPSIMD's queue.
"""

import numpy as np


def _ensure_path():
    try:
        import concourse.bass_utils  # noqa: F401
    except ImportError:
        import sys
        for p in ("/opt/trn_rl_repo", "/root/.axon_site/_ro/trn_rl_repo"):
            if p not in sys.path:
                sys.path.insert(0, p)
        import concourse.bass_utils  # noqa: F401


_ensure_path()

import ml_dtypes  # noqa: E402
import concourse.bacc as bacc  # noqa: E402
import concourse.tile as tile  # noqa: E402
from concourse import mybir  # noqa: E402
from concourse.bass_utils import run_bass_kernel_spmd  # noqa: E402
from concourse.masks import make_identity  # noqa: E402

B, T, D = 8, 4096, 2048
L = 127                      # valid timesteps per block (slot 127 = carry)
NBLK = (T + L - 1) // L      # 33
NCHUNK = D // 128
NCORES = 8
NCHAIN = 4
F32 = mybir.dt.float32
BF16 = mybir.dt.bfloat16
AF = mybir.ActivationFunctionType
ALU = mybir.AluOpType


def _chain_schedule():
    """Jobs grouped into rounds; each job: (dram_block, has_out)."""
    starts = [0, 9, 17, 25]
    chains = []
    for j, s in enumerate(starts):
        end = starts[j + 1] if j + 1 < NCHAIN else NBLK
        ch = []
        if j > 0:
            ch.append((s - 1, False))          # zero-carry warmup block
        ch += [(b, True) for b in range(s, end)]
        chains.append(ch)
    rounds = []
    r = 0
    while True:
        rnd = [(j, chains[j][r]) for j in range(NCHAIN) if r < len(chains[j])]
        if not rnd:
            break
        rounds.append(rnd)
        r += 1
    return chains, rounds


def build_kernel():
    chains, rounds = _chain_schedule()

    nc = bacc.Bacc("TRN2", target_bir_lowering=False, debug=False)
    x_d = nc.dram_tensor("x", [NBLK * 128, D], BF16,
                         kind="ExternalInput").ap()
    w2_d = nc.dram_tensor("w2", [128, 2 * NCHUNK], BF16,
                          kind="ExternalInput").ap()
    out_d = nc.dram_tensor("out", [NBLK * 128, D], BF16,
                           kind="ExternalOutput").ap()

    with tile.TileContext(nc) as tc:
        with (
            tc.tile_pool(name="consts", bufs=1) as consts,
            tc.tile_pool(name="xp", bufs=16) as xp,
            tc.tile_pool(name="xtp", bufs=6) as xtp,
            tc.tile_pool(name="op", bufs=6) as op,
            tc.tile_pool(name="gp", bufs=3) as gp,
            tc.tile_pool(name="small", bufs=12) as small,
            tc.tile_pool(name="psT", bufs=2, space="PSUM") as psT,
            tc.tile_pool(name="psO", bufs=3, space="PSUM") as psO,
            tc.tile_pool(name="psG", bufs=1, space="PSUM") as psG,
            tc.tile_pool(name="psGc", bufs=1, space="PSUM") as psGc,
            tc.tile_pool(name="psDb", bufs=1, space="PSUM") as psDb,
        ):
            identb = consts.tile([128, 128], BF16)
            make_identity(nc, identb)
            identf = consts.tile([128, 128], F32)
            make_identity(nc, identf)
            ones_row = consts.tile([1, 128], BF16)
            nc.vector.memset(ones_row, 1.0)
            w2 = consts.tile([128, 2 * NCHUNK], BF16)
            nc.sync.dma_start(out=w2, in_=w2_d)

            # per-job x tiles keyed by (chain, pos)
            xt_of = {}
            evrot = [0]

            def load_x(j, p):
                """Load x tile for chain j position p (idempotent)."""
                if p >= len(chains[j]) or (j, p) in xt_of:
                    return
                b = chains[j][p][0]
                t = xp.tile([128, D], BF16, tag="xa", name=f"xa{j}_{p}")
                nc.sync.dma_start(out=t, in_=x_d[b * 128:(b + 1) * 128, :])
                xt_of[(j, p)] = t

            def evict(out, in_):
                r = evrot[0] % 2
                evrot[0] += 1
                if r == 0:
                    nc.scalar.activation(out=out, in_=in_, func=AF.Copy)
                else:
                    nc.vector.tensor_copy(out=out, in_=in_)

            for j in range(NCHAIN):
                load_x(j, 0)

            for r, rnd in enumerate(rounds):
                jobs = [(j, r, b, has_out) for j, (b, has_out) in rnd]

                # ---- gate phase for this round's blocks ----
                gps = psG.tile([2, 512], F32, tag="g", name=f"gps{r}")
                xT = [None] * (NCHUNK // 2)
                for pp in range(NCHUNK // 2):
                    pT = psT.tile([128, 1024], BF16, tag="pt",
                                  name=f"pT{r}_{pp}")
                    for h in range(2):
                        c = 2 * pp + h
                        for idx, (j, p, b, has_out) in enumerate(jobs):
                            nc.tensor.transpose(
                                out=pT[:, h * 512 + idx * 128:
                                       h * 512 + idx * 128 + L],
                                in_=xt_of[(j, p)][0:L,
                                                  c * 128:(c + 1) * 128],
                                identity=identb[0:L, 0:L])
                    xT[pp] = xtp.tile([128, 1024], BF16, tag="xt",
                                      name=f"xT{r}_{pp}")
                    evict(xT[pp], pT)
                for pp in range(NCHUNK // 2):
                    for h in range(2):
                        c = 2 * pp + h
                        nc.tensor.matmul(
                            gps, lhsT=w2[:, 2 * c:2 * c + 2],
                            rhs=xT[pp][:, h * 512:(h + 1) * 512],
                            start=(c == 0), stop=(c == NCHUNK - 1))
                for idx in range(len(jobs)):
                    nc.vector.memset(
                        gps[0:2, idx * 128 + 127:idx * 128 + 128], 30.0)
                gsig = gp.tile([2, 512], BF16, tag="gsig", name=f"gsig{r}")
                nc.scalar.activation(out=gsig, in_=gps, func=AF.Sigmoid)
                dbc = psDb.tile([128, 512], F32, tag="dbc", name=f"dbc{r}")
                nc.tensor.matmul(dbc, lhsT=ones_row, rhs=gsig[0:1, :],
                                 start=True, stop=True)

                # loads for the next round (must precede this round's
                # carry DMAs into those tiles)
                for j, p, b, has_out in jobs:
                    load_x(j, p + 1)

                # ---- per-block A-matrix + value ----
                for idx, (j, p, b, has_out) in enumerate(jobs):
                    xa = xt_of[(j, p)]
                    gc_ps = psGc.tile([128, 2], F32, tag="gc",
                                      name=f"gc{r}_{idx}")
                    nc.tensor.matmul(
                        gc_ps, lhsT=gsig[:, idx * 128:(idx + 1) * 128],
                        rhs=identb[0:2, 0:2], start=True, stop=True)
                    gcols = small.tile([128, 2], F32, tag="gcols",
                                       name=f"gcols{r}_{idx}")
                    nc.scalar.activation(out=gcols, in_=gc_ps,
                                         func=AF.Copy)

                    # 128 cols: col 127 dead, enables fast weight load
                    ATaug = small.tile([128, 128], F32, tag="ATaug",
                                       name=f"ATaug{r}_{idx}")
                    nc.vector.tensor_tensor_scan(
                        out=ATaug,
                        data0=dbc[:, idx * 128:(idx + 1) * 128],
                        data1=identf,
                        initial=identf[:, 127:128],
                        op0=ALU.mult, op1=ALU.add)
                    ATm = small.tile([128, 128], BF16, tag="ATm",
                                     name=f"ATm{r}_{idx}")
                    nc.vector.tensor_scalar_mul(ATm, ATaug, gcols[:, 1:2])

                    out_sb = op.tile([128, D], BF16, tag="o",
                                     name=f"o{r}_{idx}")
                    for s in range(4):
                        sl = slice(s * 512, (s + 1) * 512)
                        ops = psO.tile([128, 512], F32, tag="psO",
                                       name=f"ops{r}_{idx}_{s}")
                        nc.tensor.matmul(ops, lhsT=ATm, rhs=xa[:, sl],
                                         start=True, stop=True)
                        evict(out_sb[:, sl], ops)
                    if has_out:
                        nc.sync.dma_start(
                            out=out_d[b * 128:(b + 1) * 128, :],
                            in_=out_sb)
                    # carry to the next block in this chain
                    if p + 1 < len(chains[j]):
                        nc.gpsimd.dma_start(
                            out=xt_of[(j, p + 1)][127:128, :],
                            in_=out_sb[L - 1:L, :])
    nc.compile()
    return nc


def pack_w2(w_mass, w_decay):
    w2 = np.empty((128, 2 * NCHUNK), dtype=ml_dtypes.bfloat16)
    wm = np.asarray(w_mass, np.float32).reshape(NCHUNK, 128)
    wd = np.asarray(w_decay, np.float32).reshape(NCHUNK, 128)
    w2[:, 0::2] = wd.T.astype(ml_dtypes.bfloat16)
    w2[:, 1::2] = wm.T.astype(ml_dtypes.bfloat16)
    return np.ascontiguousarray(w2)


def pad_x(xi):
    """[T, D] fp32 -> [NBLK*128, D] bf16 block-slot layout (row 127 = 0)."""
    xb = np.zeros((NBLK * 128, D), dtype=ml_dtypes.bfloat16)
    flat = np.zeros((NBLK * L, D), dtype=ml_dtypes.bfloat16)
    flat[0:T] = xi.astype(ml_dtypes.bfloat16)
    xb.reshape(NBLK, 128, D)[:, 0:L, :] = flat.reshape(NBLK, L, D)
    return xb


_CACHE = {}


def _get_nc():
    if "nc" not in _CACHE:
        _CACHE["nc"] = build_kernel()
    return _CACHE["nc"]


def make_in_maps(x, w_mass, w_decay):
    x = np.asarray(x, np.float32)
    w2 = pack_w2(w_mass, w_decay)
    return [{"x": pad_x(x[i]), "w2": w2} for i in range(B)]


def post_process(res):
    outs = []
    for i in range(B):
        o = res.results[i]["out"].reshape(NBLK, 128, D)[:, 0:L, :]
        outs.append(o.reshape(NBLK * L, D)[0:T].astype(np.float32))
    return np.stack(outs, axis=0)


def kernel(x, w_mass, w_decay):
    nc = _get_nc()
    in_maps = make_in_maps(x, w_mass, w_decay)
    res = run_bass_kernel_spmd(nc, in_maps, core_ids=list(range(NCORES)))
    return post_process(res)
